# revision 27
# baseline (speedup 1.0000x reference)
"""Multi-head attention (B=2, T=4096, C=768, H=12, Dk=64) on 8 trn2 NeuronCores.

Sharding: core c -> batch b = c//4, head-group g = c%4 (3 heads each).
Megatron-style: each core computes qkv projection for its 3 heads, full
attention for those heads, and a row-parallel partial of the output
projection. Host sums the 4 partials per batch.

v4 design notes (vs v3):
  - S^T matmuls issued as CONCURRENT row-tile pairs: head0 on PE rows
    0:63 (tile T0) and head1 on rows 64:127 (tile T8) back-to-back into
    two separate PSUM banks of one [128,2,512] tile -- the hardware
    runs 64-row tiles on disjoint row groups concurrently, doubling S
    throughput vs the v3 same-head sequential pairs.  Head 2 pairs its
    even k-blocks (rows 0:63, qkT slots 3|2) with odd k-blocks (rows
    64:127, slots 2|3) the same way.
  - one producer pipeline: per slot one ST-pair tile; exp on either
    Scalar ACT (exact, bias=lnS scale=1/8) or DVE (one-instruction
    i16-domain Schraudolph) per a tunable slot assignment, sized so
    both engines finish a q-block together.
  - softmax normalize with no DRAM round trip: DVE reciprocal reads the
    ones-row denominator straight from PSUM, GpSimd partition_broadcast
    fans it across 64 partitions, one DVE multiply writes O/den.
  - output projection contracts heads 0,1 in one 128-deep matmul: the
    normalized O^T halves are stacked into a [128,512] tile (head1 via
    a small SBUF->SBUF DMA), head2 stays a 64-deep accumulate.
  - y-projection PSUM comes from the ST-pair pool (same tag, sliced),
    so all 8 PSUM banks serve the pipeline; y jobs for q-block t pop at
    fixed slots inside q-block t+1's head-01 loop.
"""

import os
import sys
from contextlib import ExitStack

import numpy as np

for _p in ("/opt/trn_rl_repo", "/root/.axon_site/_ro/trn_rl_repo"):
    if os.path.isdir(_p) and _p not in sys.path:
        sys.path.append(_p)

import concourse.bass as bass
import concourse.mybir as mybir
import concourse.tile as tile
from concourse import bacc
from concourse.bass import ts
from concourse.bass_utils import run_bass_kernel_spmd

F32 = mybir.dt.float32
BF16 = mybir.dt.bfloat16
F16 = mybir.dt.float16
I16 = mybir.dt.int16

B, T, C = 2, 4096, 768
H, DK = 12, 64
N_CORES = 8
HPC = 3  # heads per core
GB = 512  # prologue token-block (projection free dim)
NTB = T // GB  # 8 projection blocks
GQA = 512  # attention q-block (matmul free dim)
NTQ = T // GQA  # 8 q-blocks
NTK = T // 128  # 32 tk-blocks

# one-phase Schraudolph exp (scale 1/8 folded into the affine map):
#   e = int16(s * SCH_A + SCH_B1); est = bitcast_bf16(e) ~ SCALE*exp(s/8)
# ripple ~(-1.5%, +0.6%); the ACT exact-exp tiles carry the same SCALE
# via bias=ln(SCALE) so softmax normalization cancels it everywhere.
SCH_A = 16.0 / 0.6931471805599453  # 0.125 * 2^7 / ln2 (bf16 bit domain)
SCH_B1 = 16250.406  # 127*128 - 5.594 (ripple centering)
LN_SCALE = 0.009678890188341923  # ln of the approx's mean scale

# slots (mod DVE_PERIOD) whose exp runs on DVE instead of ACT
DVE_PERIOD = 5
DVE_PHASES = (1, 3)
AH_ACT = 2  # issue ACT-destined ST pairs this many slots early
AH_DVE = 4  # issue DVE-destined ST pairs this many slots early
AV_DELAY = 6  # lag (slots) of the boundary-critical AV accumulations

_BISECT = os.environ.get("KBISECT", "")


def _dve_slots(n):
    if "noexpoff" in _BISECT:
        return frozenset()
    return frozenset(
        s for s in range(n) if s % DVE_PERIOD in DVE_PHASES
    )


def _build_program():
    nc = bacc.Bacc("TRN2", target_bir_lowering=False, debug=False)

    xT = nc.dram_tensor("xT", [C, T], BF16, kind="ExternalInput").ap()
    wqkv = nc.dram_tensor("wqkv", [C, 640], BF16, kind="ExternalInput").ap()
    bqkv = nc.dram_tensor("bqkv", [128, 5], F32, kind="ExternalInput").ap()
    wout = nc.dram_tensor("wout", [192, C], F16, kind="ExternalInput").ap()
    ident = nc.dram_tensor("ident", [128, DK], BF16, kind="ExternalInput").ap()
    y = nc.dram_tensor("y", [T, C], F32, kind="ExternalOutput").ap()

    xT3 = xT.rearrange("(o p) t -> p o t", p=128)  # [128, 6, 4096]
    y3 = y.rearrange("(n p) e -> p n e", p=128)  # [128, 32, 768]

    with tile.TileContext(nc) as tc, ExitStack() as ctx:
        sb = ctx.enter_context(tc.tile_pool(name="persist", bufs=1))

        # --- weights / biases (wqkv + ident first: needed immediately) ---
        wqkv_sb = sb.tile([128, 6, 640], BF16)
        nc.sync.dma_start(wqkv_sb[:], wqkv.rearrange("(o p) c -> p o c", p=128))
        ident_sb = sb.tile([128, DK], BF16)
        nc.sync.dma_start(ident_sb[:], ident)
        bqkv_sb = sb.tile([128, 5], F32)
        nc.sync.dma_start(bqkv_sb[:], bqkv)

        # --- persistent activations ---
        # qkT slots: o0=[q0|q1] o1=[k0|k1] o2=[q2|k2] o3=[k2|q2]
        qkT = sb.tile([128, 4, T], F16)
        # V^T feature-major: slot0=[v0|v1], slot1=[v2|junk]
        vT = sb.tile([128, 2, T], BF16)
        # V token-major per tk-block per head, with ones col at [.., 64]
        vaug = sb.tile([128, NTK, HPC, 65], BF16)
        nc.gpsimd.memset(vaug[:, :, :, 64:65], 1.0)
        lnS = sb.tile([128, 1], F32)
        nc.gpsimd.memset(lnS[:], LN_SCALE)

        # --- prologue: fused qkv projections (single pass over x) ---
        with (
            tc.tile_pool(name="psA", bufs=2, space="PSUM") as psA,
            tc.tile_pool(name="psT", bufs=2, space="PSUM") as psT,
            tc.tile_pool(name="xin", bufs=2) as xin,
        ):
            for tb in range(NTB):
                xt = xin.tile([128, 6, GB], BF16, tag="xt")
                nc.sync.dma_start(xt[:], xT3[:, :, ts(tb, GB)])
                for grp in range(5):
                    ps = psA.tile([128, GB], F32, tag="proj")
                    for d in range(6):
                        nc.tensor.matmul(
                            ps[:],
                            (wqkv_sb[:, d, ts(grp, 128)]),
                            (xt[:, d, :]),
                            start=(d == 0),
                            stop=(d == 5),
                        )
                    if grp < 3:
                        nc.vector.tensor_scalar_add(
                            qkT[:, grp, ts(tb, GB)], ps[:], bqkv_sb[:, grp : grp + 1]
                        )
                    else:
                        nc.vector.tensor_scalar_add(
                            vT[:, grp - 3, ts(tb, GB)], ps[:], bqkv_sb[:, grp : grp + 1]
                        )
                # o3 = swap halves of o2 (k2|q2)
                nc.sync.dma_start(qkT[0:64, 3, ts(tb, GB)], qkT[64:128, 2, ts(tb, GB)])
                nc.sync.dma_start(qkT[64:128, 3, ts(tb, GB)], qkT[0:64, 2, ts(tb, GB)])
                # V -> token-major via PE transpose, per 128-token sub-block
                for sub in range(GB // 128):
                    blk = tb * (GB // 128) + sub
                    for h in range(HPC):
                        if h == 0:
                            src = vT[0:64, 0, ts(blk, 128)]
                            idn = ident_sb[0:64, :]
                        elif h == 1:
                            src = vT[64:128, 0, ts(blk, 128)]
                            idn = ident_sb[64:128, :]
                        else:
                            src = vT[0:64, 1, ts(blk, 128)]
                            idn = ident_sb[0:64, :]
                        pt = psT.tile([128, DK], BF16, tag="vt")
                        nc.tensor.transpose(pt[:], src, idn)
                        nc.vector.tensor_copy(vaug[:, blk, h, 0:64], pt[:])

        # weights needed only by the attention phase: queue after x loads
        wout01_sb = sb.tile([128, C], F16)
        nc.sync.dma_start(wout01_sb[:], wout[0:128, :])
        wout2_sb = sb.tile([64, C], F16)
        nc.sync.dma_start(wout2_sb[:], wout[128:192, :])

        # --- attention + output projection ---
        with (
            tc.tile_pool(name="stp", bufs=3, space="PSUM") as stp,
            tc.tile_pool(name="psO", bufs=2, space="PSUM") as psO,
            tc.tile_pool(name="estp", bufs=10) as estp,
            tc.tile_pool(name="eip", bufs=7) as eip,
            tc.tile_pool(name="otp", bufs=2) as otp,
            tc.tile_pool(name="otcp", bufs=3) as otcp,
            tc.tile_pool(name="smallp", bufs=3) as smallp,
            tc.tile_pool(name="rbp", bufs=3) as rbp,
            tc.tile_pool(name="yp", bufs=3) as yp,
            tc.tile_pool(name="dramp", bufs=6, space="DRAM") as dramp,
        ):

            def attn_loop(n_slots, emit_st_pair, consumers, events):
                """Producer/consumer pipeline for one head-pair loop.

                Per slot: one [128,2,GQA] PSUM tile holds the ST pair
                (two concurrent 64-row matmuls on PE row tiles T0/T8 in
                separate banks); exp runs on ACT or DVE per _dve_slots.
                consumers is a list of (delay, fn(slot, est)).  Slots
                step in pairs so ST pairs / AV runs stay adjacent in the
                PE queue (fewer 64-row/128-row mode switches).  events
                maps even slot -> callables (y projections and deferred
                normalize phases of the previous q-block, placed late
                enough that their DMA dependencies are already done and
                the DVE FIFO never head-of-line blocks on a round trip).
                """
                dve = _dve_slots(n_slots)
                ahead = lambda p: AH_DVE if p in dve else AH_ACT
                fire_at = {p: max(p - ahead(p), 0) for p in range(n_slots)}
                max_delay = max(d for d, _ in consumers)
                est_of = {}
                uses_left = {}
                for s2 in range(0, n_slots + max_delay + 1, 2):
                    for fn in events.get(s2, ()):
                        fn()
                    for s in (s2, s2 + 1):
                        for p in range(n_slots):
                            if fire_at[p] != s:
                                continue
                            stt = stp.tile([128, 2, GQA], F32, tag="st")
                            emit_st_pair(p, stt)
                            if p in dve:
                                e1 = eip.tile([128, 2, GQA], I16, tag="e1")
                                nc.vector.tensor_scalar(
                                    e1[:], stt[:], SCH_A, SCH_B1,
                                    mybir.AluOpType.mult, mybir.AluOpType.add,
                                )
                                est_of[p] = e1[:].bitcast(BF16)
                            else:
                                est = estp.tile([128, 2, GQA], BF16, tag="est")
                                nc.scalar.activation(
                                    est[:], stt[:],
                                    mybir.ActivationFunctionType.Exp,
                                    bias=lnS[:], scale=0.125,
                                )
                                est_of[p] = est[:]
                            uses_left[p] = len(consumers)
                    for d, fn in consumers:
                        for s in (s2, s2 + 1):
                            b = s - d
                            if 0 <= b < n_slots:
                                fn(b, est_of[b])
                                uses_left[b] -= 1
                                if uses_left[b] == 0:
                                    est_of.pop(b)

            def normalize_start(ps_o, dst):
                """Phase 1 of dst = ps_o[0:64] / ps_o[64]: two quick DVE
                PSUM copies (the numerator copy frees the accumulator
                bank immediately) and the denominator scatter into a
                [64, 8] partition-major layout.  p2 (the 64-lane-wide DVE
                reciprocal + broadcast launch) and p3 (the multiply, on
                the otherwise-empty GpSimd compute FIFO so it fires the
                moment the broadcast lands) are deferred a few slots so
                each runs only after its DMA input has already arrived."""
                den = smallp.tile([1, GQA], F32, tag="den")
                nc.vector.tensor_copy(den[:], ps_o[64:65, :])
                otc = otcp.tile([64, GQA], F32, tag="otc")
                nc.vector.tensor_copy(otc[:], ps_o[0:64, :])
                dn = dramp.tile([GQA], F32, tag="dn")
                nc.gpsimd.dma_start(dn[:], den[:])
                d64 = smallp.tile([64, GQA // 64], F32, tag="d64")
                nc.gpsimd.dma_start(d64[:], dn.rearrange("(c p) -> p c", p=64))
                r64 = smallp.tile([64, GQA // 64], F32, tag="r64")
                dn2 = dramp.tile([GQA], F32, tag="dn2")
                rb = rbp.tile([64, GQA], F32, tag="rb")

                def p2():
                    nc.vector.reciprocal(r64[:], d64[:])
                    nc.gpsimd.dma_start(
                        dn2.rearrange("(c p) -> p c", p=64), r64[:]
                    )
                    nc.gpsimd.dma_start(
                        rb[:], dn2[None, :].to_broadcast((64, GQA))
                    )

                def p3():
                    nc.gpsimd.tensor_tensor(
                        dst, otc[:], rb[:], mybir.AluOpType.mult
                    )

                return p2, p3

            def _mk_yjobs(tq_p, otA, otB):
                jobs = []
                for tsub in range(GQA // 128):
                    for nh in range(2):
                        def job(tsub=tsub, nh=nh):
                            py_t = stp.tile([128, 2, GQA], F32, tag="st")
                            py = py_t[:, 0, 0:384]
                            nc.tensor.matmul(
                                py,
                                (otA[:, ts(tsub, 128)]),
                                (wout01_sb[:, ts(nh, 384)]),
                                start=True, stop=False,
                            )
                            nc.tensor.matmul(
                                py,
                                (otB[:, ts(tsub, 128)]),
                                (wout2_sb[:, ts(nh, 384)]),
                                start=False, stop=True,
                            )
                            y_sb = yp.tile([128, 384], F32, tag="y_sb")
                            nc.vector.tensor_copy(y_sb[:], py)
                            nc.sync.dma_start(
                                y3[:, tq_p * (GQA // 128) + tsub, ts(nh, 384)],
                                y_sb[:],
                            )
                        jobs.append(job)
                return jobs

            def _sched(evmap, start, step, fns):
                for i, fn in enumerate(fns):
                    evmap.setdefault(start + i * step, []).append(fn)

            ev01_next = {}  # events for the next h01 loop (prev tq's o2 tail)
            prev_jobs = []
            for tq in range(NTQ):
                otA = otp.tile([128, GQA], F16, tag="otA")  # [O0^T; O1^T]
                otB = otp.tile([64, GQA], F16, tag="otB")  # O2^T
                tmp1 = otp.tile([64, GQA], F16, tag="tmp1")

                ev01 = ev01_next
                ev01_next = {}
                # y jobs at slots 20,22,...,34: after the previous q-block's
                # otB multiply (scheduled at slot 8) has certainly landed
                _sched(ev01, 20, 2, prev_jobs)

                # -- heads 0,1 (concurrent on PE row tiles T0/T8) --
                ps_o0 = psO.tile([128, GQA], F32, tag="ot")
                ps_o1 = psO.tile([128, GQA], F32, tag="ot")

                def st01(b, stt):
                    nc.tensor.matmul(
                        stt[:, 0, :], (qkT[0:64, 1, ts(b, 128)]),
                        (qkT[0:64, 0, ts(tq, GQA)]),
                        start=True, stop=True,
                    )
                    nc.tensor.matmul(
                        stt[:, 1, :], (qkT[64:128, 1, ts(b, 128)]),
                        (qkT[64:128, 0, ts(tq, GQA)]),
                        start=True, stop=True,
                    )

                def av0(b, est):
                    nc.tensor.matmul(
                        ps_o0[0:65, :], (vaug[:, b, 0, :]), est[:, 0, :],
                        start=(b == 0), stop=(b == NTK - 1),
                    )

                def av1(b, est):
                    nc.tensor.matmul(
                        ps_o1[0:65, :], (vaug[:, b, 1, :]), est[:, 1, :],
                        start=(b == 0), stop=(b == NTK - 1),
                    )

                attn_loop(NTK, st01, [(0, av0), (AV_DELAY, av1)], ev01)
                p2_0, p3_0 = normalize_start(ps_o0, otA[0:64, :])
                p2_1, p3_1 = normalize_start(ps_o1, tmp1[:])
                copyA = lambda: nc.sync.dma_start(otA[64:128, :], tmp1[:])

                # -- head 2 (even blocks on T0, odd blocks on T8) --
                ps_o2 = psO.tile([128, GQA], F32, tag="ot")

                def st2(g2, stt):
                    nc.tensor.matmul(
                        stt[:, 0, :], (qkT[0:64, 3, ts(2 * g2, 128)]),
                        (qkT[0:64, 2, ts(tq, GQA)]),
                        start=True, stop=True,
                    )
                    nc.tensor.matmul(
                        stt[:, 1, :], (qkT[64:128, 2, ts(2 * g2 + 1, 128)]),
                        (qkT[64:128, 3, ts(tq, GQA)]),
                        start=True, stop=True,
                    )

                def av2(g2, est):
                    nc.tensor.matmul(
                        ps_o2[0:65, :], (vaug[:, 2 * g2, 2, :]), est[:, 0, :],
                        start=(g2 == 0), stop=False,
                    )
                    nc.tensor.matmul(
                        ps_o2[0:65, :], (vaug[:, 2 * g2 + 1, 2, :]), est[:, 1, :],
                        start=False, stop=(g2 == NTK // 2 - 1),
                    )

                ev2 = {}
                _sched(ev2, 2, 2, [p2_0, p2_1])
                _sched(ev2, 8, 2, [p3_0, p3_1, copyA])
                attn_loop(NTK // 2, st2, [(AV_DELAY, av2)], ev2)
                p2_2, p3_2 = normalize_start(ps_o2, otB[:])
                if tq < NTQ - 1:
                    _sched(ev01_next, 2, 6, [p2_2, p3_2])
                else:
                    p2_2()
                    p3_2()
                prev_jobs = _mk_yjobs(tq, otA, otB)

            for job in prev_jobs:  # drain last q-block
                job()

    nc.compile()
    return nc


_PROGRAM = None


def _get_program():
    global _PROGRAM
    if _PROGRAM is None:
        _PROGRAM = _build_program()
    return _PROGRAM


def _make_in_maps(x, W_qkv, b_qkv, W_out, b_out):
    import ml_dtypes

    bf16 = ml_dtypes.bfloat16
    x = np.asarray(x, dtype=np.float32)
    W_qkv = np.asarray(W_qkv, dtype=np.float32)
    b_qkv = np.asarray(b_qkv, dtype=np.float32)
    W_out = np.asarray(W_out, dtype=np.float32)
    b_out = np.asarray(b_out, dtype=np.float32)

    global _BOUT
    _BOUT = b_out.copy()

    xT_b = [np.ascontiguousarray(x[b].T).astype(bf16) for b in range(B)]
    ident = np.vstack([np.eye(DK), np.eye(DK)]).astype(ml_dtypes.bfloat16)
    in_maps = []
    for c in range(N_CORES):
        b, g = divmod(c, 4)
        h0 = HPC * g

        def qcol(h):
            return slice(h * DK, (h + 1) * DK)

        def kcol(h):
            return slice(C + h * DK, C + (h + 1) * DK)

        def vcol(h):
            return slice(2 * C + h * DK, 2 * C + (h + 1) * DK)

        wqkv_c = np.concatenate(
            [
                W_qkv[:, qcol(h0)],
                W_qkv[:, qcol(h0 + 1)],
                W_qkv[:, kcol(h0)],
                W_qkv[:, kcol(h0 + 1)],
                W_qkv[:, qcol(h0 + 2)],
                W_qkv[:, kcol(h0 + 2)],
                W_qkv[:, vcol(h0)],
                W_qkv[:, vcol(h0 + 1)],
                W_qkv[:, vcol(h0 + 2)],
                np.zeros((C, DK), dtype=np.float32),
            ],
            axis=1,
        )
        bqkv_c = np.concatenate(
            [
                b_qkv[qcol(h0)],
                b_qkv[qcol(h0 + 1)],
                b_qkv[kcol(h0)],
                b_qkv[kcol(h0 + 1)],
                b_qkv[qcol(h0 + 2)],
                b_qkv[kcol(h0 + 2)],
                b_qkv[vcol(h0)],
                b_qkv[vcol(h0 + 1)],
                b_qkv[vcol(h0 + 2)],
                np.zeros(DK, dtype=np.float32),
            ]
        ).reshape(5, 128).T  # [128, 5]
        in_maps.append(
            {
                "xT": np.ascontiguousarray(xT_b[b]),
                "wqkv": np.ascontiguousarray(wqkv_c.astype(bf16)),
                "bqkv": np.ascontiguousarray(bqkv_c),
                "wout": np.ascontiguousarray(
                    W_out[h0 * DK : (h0 + HPC) * DK, :]
                ).astype(np.float16),
                "ident": ident.copy(),
            }
        )
    return in_maps


_BOUT = None


def _assemble(results):
    out = np.zeros((B, T, C), dtype=np.float32)
    for c in range(N_CORES):
        out[c // 4] += results[c]["y"]
    if _BOUT is not None:
        out += _BOUT
    return out


def kernel_run(inputs, trace=False):
    """Returns (full_output [B,T,C] fp32, exec_time_ns or None)."""
    nc = _get_program()
    in_maps = _make_in_maps(**inputs)
    res = run_bass_kernel_spmd(
        nc, in_maps, core_ids=list(range(N_CORES)), trace=trace
    )
    return _assemble(res.results), res.exec_time_ns


def kernel(**inputs):
    out, _ = kernel_run(inputs)
    return out


# revision 29
# speedup vs baseline: 1.1828x; 1.1828x over previous
"""Multi-head attention (B=2, T=4096, C=768, H=12, Dk=64) on 8 trn2 NeuronCores.

Sharding: core c -> batch b = c//4, head-group g = c%4 (3 heads each).
Megatron-style: each core computes qkv projection for its 3 heads, full
attention for those heads, and a row-parallel partial of the output
projection. Host sums the 4 partials per batch.

v4 design notes (vs v3):
  - S^T matmuls issued as CONCURRENT row-tile pairs: head0 on PE rows
    0:63 (tile T0) and head1 on rows 64:127 (tile T8) back-to-back into
    two separate PSUM banks of one [128,2,512] tile -- the hardware
    runs 64-row tiles on disjoint row groups concurrently, doubling S
    throughput vs the v3 same-head sequential pairs.  Head 2 pairs its
    even k-blocks (rows 0:63, qkT slots 3|2) with odd k-blocks (rows
    64:127, slots 2|3) the same way.
  - one producer pipeline: per slot one ST-pair tile; exp on either
    Scalar ACT (exact, bias=lnS scale=1/8) or DVE (one-instruction
    i16-domain Schraudolph) per a tunable slot assignment, sized so
    both engines finish a q-block together.
  - softmax normalize with no DRAM round trip: DVE reciprocal reads the
    ones-row denominator straight from PSUM, GpSimd partition_broadcast
    fans it across 64 partitions, one DVE multiply writes O/den.
  - output projection contracts heads 0,1 in one 128-deep matmul: the
    normalized O^T halves are stacked into a [128,512] tile (head1 via
    a small SBUF->SBUF DMA), head2 stays a 64-deep accumulate.
  - y-projection PSUM comes from the ST-pair pool (same tag, sliced),
    so all 8 PSUM banks serve the pipeline; y jobs for q-block t pop at
    fixed slots inside q-block t+1's head-01 loop.
"""

import os
import sys
from contextlib import ExitStack

import numpy as np

for _p in ("/opt/trn_rl_repo", "/root/.axon_site/_ro/trn_rl_repo"):
    if os.path.isdir(_p) and _p not in sys.path:
        sys.path.append(_p)

import concourse.bass as bass
import concourse.mybir as mybir
import concourse.tile as tile
from concourse import bacc
from concourse.bass import ts
from concourse.bass_utils import run_bass_kernel_spmd

F32 = mybir.dt.float32
BF16 = mybir.dt.bfloat16
F16 = mybir.dt.float16
I16 = mybir.dt.int16

B, T, C = 2, 4096, 768
H, DK = 12, 64
N_CORES = 8
HPC = 3  # heads per core
GB = 512  # prologue token-block (projection free dim)
NTB = T // GB  # 8 projection blocks
GQA = 512  # attention q-block (matmul free dim)
NTQ = T // GQA  # 8 q-blocks
NTK = T // 128  # 32 tk-blocks

# one-phase Schraudolph exp (scale 1/8 folded into the affine map):
#   e = int16(s * SCH_A + SCH_B1); est = bitcast_bf16(e) ~ SCALE*exp(s/8)
# ripple ~(-1.5%, +0.6%); the ACT exact-exp tiles carry the same SCALE
# via bias=ln(SCALE) so softmax normalization cancels it everywhere.
SCH_A = 16.0 / 0.6931471805599453  # 0.125 * 2^7 / ln2 (bf16 bit domain)
SCH_B1 = 16250.406  # 127*128 - 5.594 (ripple centering)
LN_SCALE = 0.009678890188341923  # ln of the approx's mean scale

# slots (mod DVE_PERIOD) whose exp runs on DVE instead of ACT
DVE_PERIOD = 5
DVE_PHASES = (1, 3)
AH_ACT = 2  # issue ACT-destined ST pairs this many slots early
AH_DVE = 4  # issue DVE-destined ST pairs this many slots early
AV_DELAY = 6  # lag (slots) of the boundary-critical AV accumulations

_BISECT = os.environ.get("KBISECT", "")


def _dve_slots(n):
    if "noexpoff" in _BISECT:
        return frozenset()
    return frozenset(
        s for s in range(n) if s % DVE_PERIOD in DVE_PHASES
    )


def _build_program():
    nc = bacc.Bacc("TRN2", target_bir_lowering=False, debug=False)

    xT = nc.dram_tensor("xT", [C, T], BF16, kind="ExternalInput").ap()
    wqkv = nc.dram_tensor("wqkv", [C, 640], BF16, kind="ExternalInput").ap()
    bqkv = nc.dram_tensor("bqkv", [128, 5], F32, kind="ExternalInput").ap()
    wout = nc.dram_tensor("wout", [192, C], F16, kind="ExternalInput").ap()
    ident = nc.dram_tensor("ident", [128, DK], BF16, kind="ExternalInput").ap()
    y = nc.dram_tensor("y", [T, C], F32, kind="ExternalOutput").ap()

    xT3 = xT.rearrange("(o p) t -> p o t", p=128)  # [128, 6, 4096]
    y3 = y.rearrange("(n p) e -> p n e", p=128)  # [128, 32, 768]

    with tile.TileContext(nc) as tc, ExitStack() as ctx:
        sb = ctx.enter_context(tc.tile_pool(name="persist", bufs=1))

        # --- weights / biases (wqkv + ident first: needed immediately) ---
        wqkv_sb = sb.tile([128, 6, 640], BF16)
        nc.sync.dma_start(wqkv_sb[:], wqkv.rearrange("(o p) c -> p o c", p=128))
        ident_sb = sb.tile([128, DK], BF16)
        nc.sync.dma_start(ident_sb[:], ident)
        bqkv_sb = sb.tile([128, 5], F32)
        nc.sync.dma_start(bqkv_sb[:], bqkv)

        # --- persistent activations ---
        # qkT slots: o0=[q0|q1] o1=[k0|k1] o2=[q2|k2] o3=[k2|q2]
        qkT = sb.tile([128, 4, T], F16)
        # V^T feature-major: slot0=[v0|v1], slot1=[v2|junk]
        vT = sb.tile([128, 2, T], BF16)
        # V token-major per tk-block per head, with ones col at [.., 64]
        vaug = sb.tile([128, NTK, HPC, 65], BF16)
        nc.gpsimd.memset(vaug[:, :, :, 64:65], 1.0)
        lnS = sb.tile([128, 1], F32)
        nc.gpsimd.memset(lnS[:], LN_SCALE)

        # --- prologue: fused qkv projections (single pass over x) ---
        with (
            tc.tile_pool(name="psA", bufs=2, space="PSUM") as psA,
            tc.tile_pool(name="psT", bufs=2, space="PSUM") as psT,
            tc.tile_pool(name="xin", bufs=2) as xin,
        ):
            for tb in range(NTB):
                xt = xin.tile([128, 6, GB], BF16, tag="xt")
                nc.sync.dma_start(xt[:], xT3[:, :, ts(tb, GB)])
                for grp in range(5):
                    ps = psA.tile([128, GB], F32, tag="proj")
                    for d in range(6):
                        nc.tensor.matmul(
                            ps[:],
                            (wqkv_sb[:, d, ts(grp, 128)]),
                            (xt[:, d, :]),
                            start=(d == 0),
                            stop=(d == 5),
                        )
                    if grp < 3:
                        nc.vector.tensor_scalar_add(
                            qkT[:, grp, ts(tb, GB)], ps[:], bqkv_sb[:, grp : grp + 1]
                        )
                    else:
                        nc.vector.tensor_scalar_add(
                            vT[:, grp - 3, ts(tb, GB)], ps[:], bqkv_sb[:, grp : grp + 1]
                        )
                # o3 = swap halves of o2 (k2|q2)
                nc.sync.dma_start(qkT[0:64, 3, ts(tb, GB)], qkT[64:128, 2, ts(tb, GB)])
                nc.sync.dma_start(qkT[64:128, 3, ts(tb, GB)], qkT[0:64, 2, ts(tb, GB)])
                # V -> token-major via PE transpose, per 128-token sub-block
                for sub in range(GB // 128):
                    blk = tb * (GB // 128) + sub
                    for h in range(HPC):
                        if h == 0:
                            src = vT[0:64, 0, ts(blk, 128)]
                            idn = ident_sb[0:64, :]
                        elif h == 1:
                            src = vT[64:128, 0, ts(blk, 128)]
                            idn = ident_sb[64:128, :]
                        else:
                            src = vT[0:64, 1, ts(blk, 128)]
                            idn = ident_sb[0:64, :]
                        pt = psT.tile([128, DK], BF16, tag="vt")
                        nc.tensor.transpose(pt[:], src, idn)
                        nc.vector.tensor_copy(vaug[:, blk, h, 0:64], pt[:])

        # weights needed only by the attention phase: queue after x loads
        wout01_sb = sb.tile([128, C], F16)
        nc.sync.dma_start(wout01_sb[:], wout[0:128, :])
        wout2_sb = sb.tile([64, C], F16)
        nc.sync.dma_start(wout2_sb[:], wout[128:192, :])

        # --- attention + output projection ---
        with (
            tc.tile_pool(name="stp", bufs=3, space="PSUM") as stp,
            tc.tile_pool(name="psO", bufs=2, space="PSUM") as psO,
            tc.tile_pool(name="estp", bufs=4) as estp,
            tc.tile_pool(name="eip", bufs=3) as eip,
            tc.tile_pool(name="otp", bufs=2) as otp,
            tc.tile_pool(name="otcp", bufs=3) as otcp,
            tc.tile_pool(name="smallp", bufs=3) as smallp,
            tc.tile_pool(name="rbp", bufs=3) as rbp,
            tc.tile_pool(name="yp", bufs=3) as yp,
            tc.tile_pool(name="dramp", bufs=6, space="DRAM") as dramp,
        ):

            def attn_loop(n_slots, emit_st_pair, emit_avs, side_jobs, pops):
                """Producer/consumer pipeline for one head-pair loop.

                Per slot: one [128,2,GQA] PSUM tile holds the ST pair
                (two concurrent 64-row matmuls on PE row tiles T0/T8 in
                separate banks); exp runs on ACT or DVE per _dve_slots;
                the AV consumer reads the bf16 est pair in slot order.
                Producers are issued AH_* slots early so exp latency
                hides behind the in-order AV accumulation.  side_jobs
                (y projections of the previous q-block) pop at `pops`.
                """
                dve = _dve_slots(n_slots)
                ahead = lambda p: AH_DVE if p in dve else AH_ACT
                trig = sorted(range(n_slots), key=lambda p: (max(p - ahead(p), 0), p))
                est_of = {}
                pi = 0
                for s in range(n_slots):
                    while pi < len(trig) and max(trig[pi] - ahead(trig[pi]), 0) <= s:
                        p = trig[pi]
                        pi += 1
                        stt = stp.tile([128, 2, GQA], F32, tag="st")
                        emit_st_pair(p, stt)
                        if p in dve:
                            e1 = eip.tile([128, 2, GQA], I16, tag="e1")
                            nc.vector.tensor_scalar(
                                e1[:], stt[:], SCH_A, SCH_B1,
                                mybir.AluOpType.mult, mybir.AluOpType.add,
                            )
                            est_of[p] = e1[:].bitcast(BF16)
                        else:
                            est = estp.tile([128, 2, GQA], BF16, tag="est")
                            nc.scalar.activation(
                                est[:], stt[:],
                                mybir.ActivationFunctionType.Exp,
                                bias=lnS[:], scale=0.125,
                            )
                            est_of[p] = est[:]
                    emit_avs(s, est_of.pop(s))
                    if side_jobs and s in pops:
                        side_jobs.pop(0)()

            def normalize(ps_o, dst):
                # dst = ps_o[0:64] / ps_o[64]: bit-trick + one Newton
                # iteration reciprocal straight from PSUM (~4x cheaper on
                # DVE than InstReciprocal), GpSimd broadcast across 64
                # partitions, one DVE multiply.
                r0 = smallp.tile([1, GQA], F32, tag="r0")
                # seed: bitcast(0x7EF311C2 - bits(x)) ~ 1/x (+-5%)
                nc.vector.tensor_scalar(
                    r0[:].bitcast(mybir.dt.int32),
                    ps_o[64:65, :].bitcast(mybir.dt.int32),
                    -1, 0x7EF311C2,
                    mybir.AluOpType.mult, mybir.AluOpType.add,
                )
                t = smallp.tile([1, GQA], F32, tag="t")
                nc.vector.tensor_tensor(
                    t[:], ps_o[64:65, :], r0[:], mybir.AluOpType.mult
                )
                t2 = smallp.tile([1, GQA], F32, tag="t2")
                nc.vector.tensor_scalar(
                    t2[:], t[:], -1.0, 2.0,
                    mybir.AluOpType.mult, mybir.AluOpType.add,
                )
                rcp = smallp.tile([1, GQA], F32, tag="rcp")
                nc.vector.tensor_tensor(
                    rcp[:], r0[:], t2[:], mybir.AluOpType.mult
                )
                rb = rbp.tile([64, GQA], F32, tag="rb")
                nc.gpsimd.partition_broadcast(rb[:], rcp[:], channels=64)
                nc.vector.tensor_tensor(
                    dst, ps_o[0:64, :], rb[:], mybir.AluOpType.mult
                )

            def _mk_yjobs(tq_p, otA, otB):
                jobs = []
                for tsub in range(GQA // 128):
                    for nh in range(2):
                        def job(tsub=tsub, nh=nh):
                            py_t = stp.tile([128, 2, GQA], F32, tag="st")
                            py = py_t[:, 0, 0:384]
                            nc.tensor.matmul(
                                py,
                                (otA[:, ts(tsub, 128)]),
                                (wout01_sb[:, ts(nh, 384)]),
                                start=True, stop=False,
                            )
                            nc.tensor.matmul(
                                py,
                                (otB[:, ts(tsub, 128)]),
                                (wout2_sb[:, ts(nh, 384)]),
                                start=False, stop=True,
                            )
                            y_sb = yp.tile([128, 384], F32, tag="y_sb")
                            nc.vector.tensor_copy(y_sb[:], py)
                            nc.sync.dma_start(
                                y3[:, tq_p * (GQA // 128) + tsub, ts(nh, 384)],
                                y_sb[:],
                            )
                        jobs.append(job)
                return jobs

            H01_POPS = frozenset(range(6, 30, 3))  # 8 pops in the h01 loop
            prev_ot = None
            for tq in range(NTQ):
                otA = otp.tile([128, GQA], F16, tag="otA")  # [O0^T; O1^T]
                otB = otp.tile([64, GQA], F16, tag="otB")  # O2^T
                tmp1 = otp.tile([64, GQA], F16, tag="tmp1")
                side_jobs = _mk_yjobs(*prev_ot) if prev_ot else []

                # -- heads 0,1 (concurrent on PE row tiles T0/T8) --
                ps_o0 = psO.tile([128, GQA], F32, tag="ot")
                ps_o1 = psO.tile([128, GQA], F32, tag="ot")

                def st01(b, stt):
                    nc.tensor.matmul(
                        stt[:, 0, :], (qkT[0:64, 1, ts(b, 128)]),
                        (qkT[0:64, 0, ts(tq, GQA)]),
                        start=True, stop=True,
                    )
                    nc.tensor.matmul(
                        stt[:, 1, :], (qkT[64:128, 1, ts(b, 128)]),
                        (qkT[64:128, 0, ts(tq, GQA)]),
                        start=True, stop=True,
                    )

                def av01(b, est):
                    nc.tensor.matmul(
                        ps_o0[0:65, :], (vaug[:, b, 0, :]), est[:, 0, :],
                        start=(b == 0), stop=(b == NTK - 1),
                    )
                    nc.tensor.matmul(
                        ps_o1[0:65, :], (vaug[:, b, 1, :]), est[:, 1, :],
                        start=(b == 0), stop=(b == NTK - 1),
                    )

                attn_loop(NTK, st01, av01, side_jobs, H01_POPS)
                normalize(ps_o0, otA[0:64, :])
                normalize(ps_o1, tmp1[:])
                nc.gpsimd.dma_start(otA[64:128, :], tmp1[:])

                # -- head 2 (even blocks on T0, odd blocks on T8) --
                ps_o2 = psO.tile([128, GQA], F32, tag="ot")

                def st2(g2, stt):
                    nc.tensor.matmul(
                        stt[:, 0, :], (qkT[0:64, 3, ts(2 * g2, 128)]),
                        (qkT[0:64, 2, ts(tq, GQA)]),
                        start=True, stop=True,
                    )
                    nc.tensor.matmul(
                        stt[:, 1, :], (qkT[64:128, 2, ts(2 * g2 + 1, 128)]),
                        (qkT[64:128, 3, ts(tq, GQA)]),
                        start=True, stop=True,
                    )

                def av2(g2, est):
                    nc.tensor.matmul(
                        ps_o2[0:65, :], (vaug[:, 2 * g2, 2, :]), est[:, 0, :],
                        start=(g2 == 0), stop=False,
                    )
                    nc.tensor.matmul(
                        ps_o2[0:65, :], (vaug[:, 2 * g2 + 1, 2, :]), est[:, 1, :],
                        start=False, stop=(g2 == NTK // 2 - 1),
                    )

                attn_loop(NTK // 2, st2, av2, side_jobs, frozenset())
                normalize(ps_o2, otB[:])
                prev_ot = (tq, otA, otB)

            for job in _mk_yjobs(*prev_ot):  # drain last q-block
                job()

    nc.compile()
    return nc


_PROGRAM = None


def _get_program():
    global _PROGRAM
    if _PROGRAM is None:
        _PROGRAM = _build_program()
    return _PROGRAM


def _make_in_maps(x, W_qkv, b_qkv, W_out, b_out):
    import ml_dtypes

    bf16 = ml_dtypes.bfloat16
    x = np.asarray(x, dtype=np.float32)
    W_qkv = np.asarray(W_qkv, dtype=np.float32)
    b_qkv = np.asarray(b_qkv, dtype=np.float32)
    W_out = np.asarray(W_out, dtype=np.float32)
    b_out = np.asarray(b_out, dtype=np.float32)

    global _BOUT
    _BOUT = b_out.copy()

    xT_b = [np.ascontiguousarray(x[b].T).astype(bf16) for b in range(B)]
    ident = np.vstack([np.eye(DK), np.eye(DK)]).astype(ml_dtypes.bfloat16)
    in_maps = []
    for c in range(N_CORES):
        b, g = divmod(c, 4)
        h0 = HPC * g

        def qcol(h):
            return slice(h * DK, (h + 1) * DK)

        def kcol(h):
            return slice(C + h * DK, C + (h + 1) * DK)

        def vcol(h):
            return slice(2 * C + h * DK, 2 * C + (h + 1) * DK)

        wqkv_c = np.concatenate(
            [
                W_qkv[:, qcol(h0)],
                W_qkv[:, qcol(h0 + 1)],
                W_qkv[:, kcol(h0)],
                W_qkv[:, kcol(h0 + 1)],
                W_qkv[:, qcol(h0 + 2)],
                W_qkv[:, kcol(h0 + 2)],
                W_qkv[:, vcol(h0)],
                W_qkv[:, vcol(h0 + 1)],
                W_qkv[:, vcol(h0 + 2)],
                np.zeros((C, DK), dtype=np.float32),
            ],
            axis=1,
        )
        bqkv_c = np.concatenate(
            [
                b_qkv[qcol(h0)],
                b_qkv[qcol(h0 + 1)],
                b_qkv[kcol(h0)],
                b_qkv[kcol(h0 + 1)],
                b_qkv[qcol(h0 + 2)],
                b_qkv[kcol(h0 + 2)],
                b_qkv[vcol(h0)],
                b_qkv[vcol(h0 + 1)],
                b_qkv[vcol(h0 + 2)],
                np.zeros(DK, dtype=np.float32),
            ]
        ).reshape(5, 128).T  # [128, 5]
        in_maps.append(
            {
                "xT": np.ascontiguousarray(xT_b[b]),
                "wqkv": np.ascontiguousarray(wqkv_c.astype(bf16)),
                "bqkv": np.ascontiguousarray(bqkv_c),
                "wout": np.ascontiguousarray(
                    W_out[h0 * DK : (h0 + HPC) * DK, :]
                ).astype(np.float16),
                "ident": ident.copy(),
            }
        )
    return in_maps


_BOUT = None


def _assemble(results):
    out = np.zeros((B, T, C), dtype=np.float32)
    for c in range(N_CORES):
        out[c // 4] += results[c]["y"]
    if _BOUT is not None:
        out += _BOUT
    return out


def kernel_run(inputs, trace=False):
    """Returns (full_output [B,T,C] fp32, exec_time_ns or None)."""
    nc = _get_program()
    in_maps = _make_in_maps(**inputs)
    res = run_bass_kernel_spmd(
        nc, in_maps, core_ids=list(range(N_CORES)), trace=trace
    )
    return _assemble(res.results), res.exec_time_ns


def kernel(**inputs):
    out, _ = kernel_run(inputs)
    return out
